# revision 5
# baseline (speedup 1.0000x reference)
"""Trainium2 Bass kernel for AdaptiveFeatureSelector (topk_masking).

v2: group-software-pipelined single pass.
 - Selector nets in 3-term fp16 split matmuls (hi/lo weights + dual-h /
   x-lo corrections) -> c = sigmoid*sigmoid in f32.
 - Per-row exact top-K=358: PE-transpose c to row-major (f32), per-row
   mean accumulated by the ACT eviction pass, affine-calibrated center
   t0, fp16 residuals, ITERS-step bisection with per-column counts
   (DVE tensor_scalar 4x + one ACT Sign column per group).
 - Mask = (resid >= lo), PE-transposed back, applied to xh, recon MLP,
   fp16 output (+br2 on device), un-transposed on host.
Phases are interleaved by group (A: selector+residuals, B: bisection,
C: mask+recon) so PE/ACT/DVE overlap.
"""

import sys

sys.path.insert(0, "/opt/trn_rl_repo")
import numpy as np

D = 512
H = 128
K = 358
B = 65536
NCORES = 8
R = B // NCORES
CHUNK = 512
NCHUNK = R // CHUNK      # 16
NCOL = R // 128          # 64
NG = 4
GC = NCOL // NG          # 16 cols per group
CPG = NCHUNK // NG       # 4 chunks per group
W_WIN = 0.015
ITERS = 12
NA = 0                   # ACT-counted cols per group (rest DVE)
OSPLIT = [(0, 0, 128), (1, 128, 128), (2, 256, 102)]

_cache = {}


def _f16(a):
    return np.asarray(a, np.float16)


def _split16(a):
    hi = _f16(a)
    lo = _f16(np.asarray(a, np.float32) - hi.astype(np.float32))
    return hi, lo


def _sig(a):
    return 1.0 / (1.0 + np.exp(-a))


def _calibrate(x, P):
    """Simulate the device c-pipeline on 512 rows; fit thr ~ A*mu + C."""
    xs = np.asarray(x[:512], np.float32)
    xh = _f16(xs)
    xl = _f16(xs - xh.astype(np.float32))

    def mm3(ah, al, Wm):
        wh, wl = _split16(Wm)
        out = ah.astype(np.float32) @ wh.astype(np.float32)
        out = out + ah.astype(np.float32) @ wl.astype(np.float32)
        if al is not None:
            out = out + al.astype(np.float32) @ wh.astype(np.float32)
        return out

    def ev(a):
        h = np.maximum(a, 0)
        hh = _f16(h)
        return hh, _f16(h - hh.astype(np.float32))

    h1h, h1l = ev(mm3(xh, xl, P["W1"]) + P["b1"])
    h2h, h2l = ev(mm3(h1h, h1l, P["W2"]) + P["b2"])
    imp = _sig(mm3(h2h, h2l, P["W3"]) + P["b3"])
    g1h, g1l = ev(mm3(xh, xl, P["Wg1"]) + P["bg1"])
    gate = _sig(mm3(g1h, g1l, P["Wg2"]) + P["bg2"])
    c = (imp * gate).astype(np.float32)
    mu = c.mean(1)
    thr = np.partition(c, D - K, axis=1)[:, D - K]
    A1, C1 = np.polyfit(mu, thr, 1)
    return float(A1), float(C1)


def _build_program():
    from concourse import bacc, mybir, tile

    f32 = mybir.dt.float32
    fp16 = mybir.dt.float16
    Act = mybir.ActivationFunctionType
    Alu = mybir.AluOpType

    nc = bacc.Bacc("TRN2", target_bir_lowering=False, debug=False,
                   num_devices=NCORES)

    def din(name, shape, dt=fp16):
        return nc.dram_tensor(name, shape, dt, kind="ExternalInput").ap()

    xh_d = din("xh", [4, 128, R])
    xl_d = din("xl", [4, 128, R])
    w1h_d = din("w1h", [4, 128, 128]); w1l_d = din("w1l", [4, 128, 128])
    w2h_d = din("w2h", [128, 128]);    w2l_d = din("w2l", [128, 128])
    w3h_d = din("w3h", [4, 128, 128]); w3l_d = din("w3l", [4, 128, 128])
    wg1h_d = din("wg1h", [4, 128, 128]); wg1l_d = din("wg1l", [4, 128, 128])
    wg2h_d = din("wg2h", [4, 128, 128]); wg2l_d = din("wg2l", [4, 128, 128])
    wr1_d = din("wr1", [4, 128, 128])
    wr2_d = din("wr2", [128, K])
    b1_d = din("b1", [128, 1], f32)
    b2_d = din("b2", [128, 1], f32)
    b3_d = din("b3", [128, 4], f32)
    bg1_d = din("bg1", [128, 1], f32)
    bg2_d = din("bg2", [128, 4], f32)
    br1_d = din("br1", [128, 1], f32)
    br2_d = din("br2", [128, 3], f32)
    coef_d = din("coef", [128, 2], f32)
    idf32_d = din("idf32", [128, 128], f32)
    idf16_d = din("idf16", [128, 128], fp16)
    out_d = nc.dram_tensor("out", [3, 128, R], fp16, kind="ExternalOutput").ap()

    with tile.TileContext(nc) as tc:
        with (
            tc.tile_pool(name="wts", bufs=1) as wts,
            tc.tile_pool(name="big", bufs=1) as big,
            tc.tile_pool(name="xls", bufs=2) as xls,
            tc.tile_pool(name="hbuf", bufs=2) as hbuf,
            tc.tile_pool(name="sgbuf", bufs=1) as sgbuf,
            tc.tile_pool(name="crmb", bufs=2) as crmb,
            tc.tile_pool(name="mkb", bufs=2) as mkb,
            tc.tile_pool(name="rrb", bufs=2) as rrb,
            tc.tile_pool(name="obuf", bufs=2) as obuf,
            tc.tile_pool(name="st", bufs=1) as st,
            tc.tile_pool(name="ps_h", bufs=2, space="PSUM") as ps_h,
            tc.tile_pool(name="ps_s", bufs=2, space="PSUM") as ps_s,
            tc.tile_pool(name="ps_t", bufs=2, space="PSUM") as ps_t,
            tc.tile_pool(name="ps_m", bufs=2, space="PSUM") as ps_m,
        ):
            def ldt(dram, tiles, tag, dt=fp16):
                t = wts.tile([128, tiles, 128], dt, tag=tag)
                nc.sync.dma_start(t, dram.rearrange("t p m -> p t m"))
                return t

            def ld2(dram, shape, tag, dt=f32):
                t = wts.tile(shape, dt, tag=tag)
                nc.sync.dma_start(t, dram)
                return t

            w1h = ldt(w1h_d, 4, "w1h"); w1l = ldt(w1l_d, 4, "w1l")
            w2h = ld2(w2h_d, [128, 128], "w2h", fp16)
            w2l = ld2(w2l_d, [128, 128], "w2l", fp16)
            w3h = ldt(w3h_d, 4, "w3h"); w3l = ldt(w3l_d, 4, "w3l")
            wg1h = ldt(wg1h_d, 4, "wg1h"); wg1l = ldt(wg1l_d, 4, "wg1l")
            wg2h = ldt(wg2h_d, 4, "wg2h"); wg2l = ldt(wg2l_d, 4, "wg2l")
            wr1 = ldt(wr1_d, 4, "wr1")
            wr2 = ld2(wr2_d, [128, K], "wr2", fp16)
            b1 = ld2(b1_d, [128, 1], "b1"); b2 = ld2(b2_d, [128, 1], "b2")
            b3 = ld2(b3_d, [128, 4], "b3")
            bg1 = ld2(bg1_d, [128, 1], "bg1")
            bg2 = ld2(bg2_d, [128, 4], "bg2")
            br1 = ld2(br1_d, [128, 1], "br1")
            br2 = ld2(br2_d, [128, 3], "br2")
            coef = ld2(coef_d, [128, 2], "coef")
            idf32 = ld2(idf32_d, [128, 128], "idf32")
            idf16 = ld2(idf16_d, [128, 128], "idf16", fp16)

            xh = big.tile([128, 4, R], fp16, tag="xh")
            nc.sync.dma_start(xh, xh_d.rearrange("f p r -> p f r"))
            ebuf = big.tile([128, NCOL, 512], fp16, tag="ebuf")
            junkD = big.tile([128, 512], fp16, tag="junkD")
            junkA = big.tile([128, 512], fp16, tag="junkA")

            lo_g, hi_g, tmp_g, cnt_g, gek_g, gekn_g, ng_g = [], [], [], [], [], [], []
            t0n = st.tile([128, NCOL], f32, tag="t0n")
            mus = st.tile([128, NCOL], f32, tag="mus")
            u32 = mybir.dt.uint32
            for g in range(NG):
                for lst, nm, dt_ in (
                        (lo_g, "lo", f32), (hi_g, "hi", f32),
                        (tmp_g, "tmp", f32), (cnt_g, "cnt", f32),
                        (gek_g, "gek", u32), (gekn_g, "gekn", u32),
                        (ng_g, "ng", f32)):
                    lst.append(st.tile([128, GC], dt_, tag="%s%d" % (nm, g),
                                       name="%s%d" % (nm, g)))

            def evict_dual(psum, bias, dual):
                """relu(psum)+bias via ACT (hh) and DVE residual (hl)."""
                hh = hbuf.tile([128, CHUNK], fp16, tag="hh")
                nc.scalar.activation(hh, psum, Act.Relu, bias=bias)
                if not dual:
                    return hh, None
                hl = hbuf.tile([128, CHUNK], fp16, tag="hl")
                nc.vector.scalar_tensor_tensor(
                    hl, psum, 0.0, hh, op0=Alu.max, op1=Alu.subtract)
                return hh, hl

            def net3(stats, movs, psum):
                ops = []
                for (sh_, sl_), (mh, ml) in zip(stats, movs):
                    ops.append((sh_, mh))
                    if ml is not None:
                        ops.append((sh_, ml))
                    ops.append((sl_, mh))
                n = len(ops)
                for i, (sta, mov) in enumerate(ops):
                    nc.tensor.matmul(psum, lhsT=sta, rhs=mov,
                                     start=(i == 0), stop=(i == n - 1))

            # =============== phase A ===============
            def phase_a(ck):
                r0 = ck * CHUNK
                xhc = [xh[:, ft, r0:r0 + CHUNK] for ft in range(4)]
                xlt = xls.tile([128, 4, CHUNK], fp16, tag="xl")
                for ft in range(4):
                    nc.sync.dma_start(xlt[:, ft, :], xl_d[ft, :, r0:r0 + CHUNK])
                xmov = [(xhc[ft], xlt[:, ft, :]) for ft in range(4)]

                p = ps_h.tile([128, CHUNK], f32, tag="h")
                net3([(w1h[:, ki, :], w1l[:, ki, :]) for ki in range(4)],
                     xmov, p)
                h1h, h1l = evict_dual(p, b1, True)

                p = ps_h.tile([128, CHUNK], f32, tag="h")
                net3([(w2h, w2l)], [(h1h, h1l)], p)
                h2h, h2l = evict_dual(p, b2, True)

                sa = []
                for mt in range(4):
                    pw = ps_s.tile([128, CHUNK], f32, tag="s")
                    net3([(w3h[:, mt, :], w3l[:, mt, :])], [(h2h, h2l)], pw)
                    t = sgbuf.tile([128, CHUNK], f32, tag="sa%d" % mt,
                                   name="sa%d" % mt)
                    nc.scalar.activation(t, pw, Act.Sigmoid,
                                         bias=b3[:, mt:mt + 1])
                    sa.append(t)

                p = ps_h.tile([128, CHUNK], f32, tag="h")
                net3([(wg1h[:, ki, :], wg1l[:, ki, :]) for ki in range(4)],
                     xmov, p)
                g1h, g1l = evict_dual(p, bg1, True)

                ct = []
                for mt in range(4):
                    pw = ps_s.tile([128, CHUNK], f32, tag="s")
                    net3([(wg2h[:, mt, :], wg2l[:, mt, :])], [(g1h, g1l)], pw)
                    t = sgbuf.tile([128, CHUNK], f32, tag="sg%d" % (mt % 2),
                                   name="sg%d" % (mt % 2))
                    nc.scalar.activation(t, pw, Act.Sigmoid,
                                         bias=bg2[:, mt:mt + 1])
                    c = sgbuf.tile([128, CHUNK], f32, tag="c%d" % mt,
                                   name="c%d" % mt)
                    nc.vector.tensor_mul(c, sa[mt], t)
                    ct.append(c)

                for rt in range(4):
                    col = ck * 4 + rt
                    ptr = ps_t.tile([128, CHUNK], f32, tag="tr")
                    for mt in range(4):
                        nc.tensor.transpose(
                            ptr[:, mt * 128:(mt + 1) * 128],
                            ct[mt][:, rt * 128:(rt + 1) * 128], idf32)
                    crm = crmb.tile([128, CHUNK], f32, tag="crm")
                    nc.scalar.activation(crm, ptr, Act.Identity,
                                         accum_out=mus[:, col:col + 1])
                    nc.vector.tensor_scalar(
                        t0n[:, col:col + 1], mus[:, col:col + 1],
                        coef[:, 0:1], coef[:, 1:2],
                        op0=Alu.mult, op1=Alu.add)
                    nc.vector.tensor_scalar(
                        ebuf[:, col, :], crm, t0n[:, col:col + 1], None,
                        op0=Alu.add)

            # =============== phase B ===============
            def phase_b_init(g):
                nc.vector.memset(lo_g[g], -W_WIN)
                nc.vector.memset(hi_g[g], W_WIN)

            def phase_b_iter(g, it):
                lo, hi, tmp = lo_g[g], hi_g[g], tmp_g[g]
                cnt, gek, gekn, ngt = cnt_g[g], gek_g[g], gekn_g[g], ng_g[g]
                nc.vector.tensor_add(tmp, lo, hi)
                nc.vector.tensor_scalar_mul(tmp, tmp, 0.5)   # tmp = mid
                if NA:
                    nc.vector.tensor_scalar_mul(
                        ngt[:, GC - NA:], tmp[:, GC - NA:], -1.0)
                for i in range(GC - NA):
                    col = g * GC + i
                    nc.vector.tensor_scalar(
                        junkD, ebuf[:, col, :], tmp[:, i:i + 1], 0.0,
                        op0=Alu.is_ge, op1=Alu.add,
                        accum_out=cnt[:, i:i + 1])
                for i in range(GC - NA, GC):
                    col = g * GC + i
                    nc.scalar.activation(
                        junkA, ebuf[:, col, :], Act.Sign,
                        bias=ngt[:, i:i + 1],
                        accum_out=cnt[:, i:i + 1])
                nc.vector.tensor_scalar(
                    gek[:, :GC - NA], cnt[:, :GC - NA], float(K), None,
                    op0=Alu.is_ge)
                nc.vector.tensor_scalar(
                    gekn[:, :GC - NA], cnt[:, :GC - NA], float(K), None,
                    op0=Alu.is_lt)
                if NA:
                    thv = float(2 * K - 512)
                    nc.vector.tensor_scalar(
                        gek[:, GC - NA:], cnt[:, GC - NA:], thv, None,
                        op0=Alu.is_ge)
                    nc.vector.tensor_scalar(
                        gekn[:, GC - NA:], cnt[:, GC - NA:], thv, None,
                        op0=Alu.is_lt)
                nc.vector.copy_predicated(lo, gek, tmp)
                nc.vector.copy_predicated(hi, gekn, tmp)

            # =============== phase C ===============
            def phase_c(ck):
                g = ck // CPG
                r0 = ck * CHUNK
                mk = mkb.tile([128, 4, 512], fp16, tag="mk")
                for rt in range(4):
                    col = ck * 4 + rt
                    nc.vector.tensor_scalar(
                        mk[:, rt, :], ebuf[:, col, :],
                        lo_g[g][:, col - g * GC:col - g * GC + 1], None,
                        op0=Alu.is_ge)
                masked = []
                for ft in range(4):
                    pm = ps_m.tile([128, CHUNK], fp16, tag="pm")
                    for rt in range(4):
                        nc.tensor.transpose(
                            pm[:, rt * 128:(rt + 1) * 128],
                            mk[:, rt, ft * 128:(ft + 1) * 128], idf16)
                    mkd = mkb.tile([128, CHUNK], fp16, tag="mkd")
                    nc.vector.tensor_mul(mkd, pm, xh[:, ft, r0:r0 + CHUNK])
                    masked.append(mkd)

                p = ps_h.tile([128, CHUNK], f32, tag="h")
                for ft in range(4):
                    nc.tensor.matmul(p, lhsT=wr1[:, ft, :], rhs=masked[ft],
                                     start=(ft == 0), stop=(ft == 3))
                rr = rrb.tile([128, CHUNK], fp16, tag="rr")
                nc.scalar.activation(rr, p, Act.Relu, bias=br1)

                for (ot, o0, ow) in OSPLIT:
                    po = ps_h.tile([128, CHUNK], f32, tag="h")
                    nc.tensor.matmul(po[0:ow, :], lhsT=wr2[:, o0:o0 + ow],
                                     rhs=rr, start=True, stop=True)
                    of = obuf.tile([128, CHUNK], fp16, tag="of")
                    nc.vector.tensor_scalar(
                        of[0:ow, :], po[0:ow, :], br2[0:ow, ot:ot + 1], None,
                        op0=Alu.add)
                    nc.sync.dma_start(out_d[ot, 0:ow, r0:r0 + CHUNK],
                                      of[0:ow, :])

            # =============== emission (software pipeline) ===============
            IT_SL = [(ITERS * s // CPG, ITERS * (s + 1) // CPG)
                     for s in range(CPG)]
            for g in range(NG):
                phase_b_init(g)
            for ck in range(NCHUNK):
                g, sl = ck // CPG, ck % CPG
                phase_a(ck)
                if g >= 1:
                    for it in range(*IT_SL[sl]):
                        phase_b_iter(g - 1, it)
                if g >= 2:
                    phase_c(CPG * (g - 2) + sl)
            for sl in range(CPG):
                for it in range(*IT_SL[sl]):
                    phase_b_iter(NG - 1, it)
                phase_c(CPG * (NG - 2) + sl)
            for sl in range(CPG):
                phase_c(CPG * (NG - 1) + sl)

    nc.compile()
    return nc


def kernel(**inputs):
    from concourse.bass_utils import run_bass_kernel_spmd

    x = np.asarray(inputs["x"], np.float32)
    names = ["W1", "b1", "W2", "b2", "W3", "b3", "Wg1", "bg1", "Wg2", "bg2",
             "Wr1", "br1", "Wr2", "br2"]
    P = {n: np.asarray(inputs[n], np.float32) for n in names}

    A1, C1 = _calibrate(x, P)

    def ksplit(Wm):
        h, l = _split16(Wm)
        return (np.ascontiguousarray(h.reshape(4, 128, 128)),
                np.ascontiguousarray(l.reshape(4, 128, 128)))

    def msplit(Wm):
        h, l = _split16(Wm)
        return (np.ascontiguousarray(h.reshape(128, 4, 128).transpose(1, 0, 2)),
                np.ascontiguousarray(l.reshape(128, 4, 128).transpose(1, 0, 2)))

    w1h, w1l = ksplit(P["W1"])
    w2h, w2l = _split16(P["W2"])
    w3h, w3l = msplit(P["W3"])
    wg1h, wg1l = ksplit(P["Wg1"])
    wg2h, wg2l = msplit(P["Wg2"])
    wr1 = np.ascontiguousarray(_f16(P["Wr1"]).reshape(4, 128, 128))
    wr2 = np.ascontiguousarray(_f16(P["Wr2"]))
    coef = np.zeros((128, 2), np.float32)
    coef[:, 0] = -A1 / 512.0
    coef[:, 1] = -C1
    b3p = np.zeros((128, 4), np.float32)
    b3p[:] = P["b3"].reshape(4, 128).T
    bg2p = np.zeros((128, 4), np.float32)
    bg2p[:] = P["bg2"].reshape(4, 128).T
    br2t = np.zeros(384, np.float32)
    br2t[:K] = P["br2"]
    br2p = np.ascontiguousarray(br2t.reshape(3, 128).T)
    ident = np.eye(128)
    shared = dict(
        w1h=w1h, w1l=w1l,
        w2h=np.ascontiguousarray(w2h), w2l=np.ascontiguousarray(w2l),
        w3h=w3h, w3l=w3l,
        wg1h=wg1h, wg1l=wg1l, wg2h=wg2h, wg2l=wg2l,
        wr1=wr1, wr2=wr2,
        b1=P["b1"].reshape(128, 1), b2=P["b2"].reshape(128, 1),
        b3=b3p, bg1=P["bg1"].reshape(128, 1), bg2=bg2p,
        br1=P["br1"].reshape(128, 1), br2=br2p,
        coef=coef,
        idf32=ident.astype(np.float32),
        idf16=ident.astype(np.float16),
    )

    in_maps = []
    for i in range(NCORES):
        xs = x[i * R:(i + 1) * R]
        xT = np.ascontiguousarray(xs.T)
        xTh = _f16(xT)
        xTl = _f16(xT - xTh.astype(np.float32))
        m = dict(shared)
        m["xh"] = np.ascontiguousarray(xTh.reshape(4, 128, R))
        m["xl"] = np.ascontiguousarray(xTl.reshape(4, 128, R))
        in_maps.append(m)

    if "nc" not in _cache:
        _cache["nc"] = _build_program()
    nc = _cache["nc"]
    _cache["in_maps"] = in_maps

    res = run_bass_kernel_spmd(nc, in_maps, list(range(NCORES)))
    outs = []
    for i in range(NCORES):
        o = res.results[i]["out"].astype(np.float32)   # [3,128,R]
        o = o.reshape(384, R)[:K]
        outs.append(np.ascontiguousarray(o.T))
    return np.concatenate(outs, axis=0)


if __name__ == "__main__":
    rng = np.random.default_rng(0)
    fake = {"x": rng.standard_normal((B, D), dtype=np.float32)}
    s = lambda f: 1.0 / np.sqrt(f)
    for nm, sh, fan in [("W1", (D, H), D), ("W2", (H, H), H), ("W3", (H, D), H),
                        ("Wg1", (D, H), D), ("Wg2", (H, D), H),
                        ("Wr1", (D, H), D), ("Wr2", (H, K), H)]:
        fake[nm] = rng.uniform(-s(fan), s(fan), sh).astype(np.float32)
    for nm, sh in [("b1", H), ("b2", H), ("b3", D), ("bg1", H), ("bg2", D),
                   ("br1", H), ("br2", K)]:
        fake[nm] = np.zeros(sh, np.float32)
    out = kernel(**fake)
    print("out", out.shape, out.dtype, float(np.abs(out).max()))


# revision 6
# speedup vs baseline: 1.1143x; 1.1143x over previous
"""Trainium2 Bass kernel for AdaptiveFeatureSelector (topk_masking).

v2: group-software-pipelined single pass.
 - Selector nets in 3-term fp16 split matmuls (hi/lo weights + dual-h /
   x-lo corrections) -> c = sigmoid*sigmoid in f32.
 - Per-row exact top-K=358: PE-transpose c to row-major (f32), per-row
   mean accumulated by the ACT eviction pass, affine-calibrated center
   t0, fp16 residuals, ITERS-step bisection with per-column counts
   (DVE tensor_scalar 4x + one ACT Sign column per group).
 - Mask = (resid >= lo), PE-transposed back, applied to xh, recon MLP,
   fp16 output (+br2 on device), un-transposed on host.
Phases are interleaved by group (A: selector+residuals, B: bisection,
C: mask+recon) so PE/ACT/DVE overlap.
"""

import sys

sys.path.insert(0, "/opt/trn_rl_repo")
import numpy as np

D = 512
H = 128
K = 358
B = 65536
NCORES = 8
R = B // NCORES
CHUNK = 512
NCHUNK = R // CHUNK      # 16
NCOL = R // 128          # 64
NG = 4
GC = NCOL // NG          # 16 cols per group
CPG = NCHUNK // NG       # 4 chunks per group
W_WIN = 0.015
ITERS = 12
NA = 2                   # ACT-counted cols per group (rest DVE)
OSPLIT = [(0, 0, 128), (1, 128, 128), (2, 256, 102)]

_cache = {}


def _f16(a):
    return np.asarray(a, np.float16)


def _split16(a):
    hi = _f16(a)
    lo = _f16(np.asarray(a, np.float32) - hi.astype(np.float32))
    return hi, lo


def _sig(a):
    return 1.0 / (1.0 + np.exp(-a))


def _calibrate(x, P):
    """Simulate the device c-pipeline on 512 rows; fit thr ~ A*mu + C."""
    xs = np.asarray(x[:512], np.float32)
    xh = _f16(xs)
    xl = _f16(xs - xh.astype(np.float32))

    def mm3(ah, al, Wm):
        wh, wl = _split16(Wm)
        out = ah.astype(np.float32) @ wh.astype(np.float32)
        out = out + ah.astype(np.float32) @ wl.astype(np.float32)
        if al is not None:
            out = out + al.astype(np.float32) @ wh.astype(np.float32)
        return out

    def ev(a):
        h = np.maximum(a, 0)
        hh = _f16(h)
        return hh, _f16(h - hh.astype(np.float32))

    h1h, h1l = ev(mm3(xh, xl, P["W1"]) + P["b1"])
    h2h, h2l = ev(mm3(h1h, h1l, P["W2"]) + P["b2"])
    imp = _sig(mm3(h2h, h2l, P["W3"]) + P["b3"])
    g1h, g1l = ev(mm3(xh, xl, P["Wg1"]) + P["bg1"])
    gate = _sig(mm3(g1h, g1l, P["Wg2"]) + P["bg2"])
    c = (imp * gate).astype(np.float32)
    mu = c.mean(1)
    thr = np.partition(c, D - K, axis=1)[:, D - K]
    A1, C1 = np.polyfit(mu, thr, 1)
    return float(A1), float(C1)


def _build_program():
    from concourse import bacc, mybir, tile

    f32 = mybir.dt.float32
    fp16 = mybir.dt.float16
    Act = mybir.ActivationFunctionType
    Alu = mybir.AluOpType

    nc = bacc.Bacc("TRN2", target_bir_lowering=False, debug=False,
                   num_devices=NCORES)

    def din(name, shape, dt=fp16):
        return nc.dram_tensor(name, shape, dt, kind="ExternalInput").ap()

    xh_d = din("xh", [4, 128, R])
    xl_d = din("xl", [4, 128, R])
    w1h_d = din("w1h", [4, 128, 128]); w1l_d = din("w1l", [4, 128, 128])
    w2h_d = din("w2h", [128, 128]);    w2l_d = din("w2l", [128, 128])
    w3h_d = din("w3h", [4, 128, 128]); w3l_d = din("w3l", [4, 128, 128])
    wg1h_d = din("wg1h", [4, 128, 128]); wg1l_d = din("wg1l", [4, 128, 128])
    wg2h_d = din("wg2h", [4, 128, 128]); wg2l_d = din("wg2l", [4, 128, 128])
    wr1_d = din("wr1", [4, 128, 128])
    wr2_d = din("wr2", [128, K])
    b1_d = din("b1", [128, 1], f32)
    b2_d = din("b2", [128, 1], f32)
    b3_d = din("b3", [128, 4], f32)
    bg1_d = din("bg1", [128, 1], f32)
    bg2_d = din("bg2", [128, 4], f32)
    br1_d = din("br1", [128, 1], f32)
    br2_d = din("br2", [128, 3], f32)
    coef_d = din("coef", [128, 2], f32)
    idf32_d = din("idf32", [128, 128], f32)
    idf16_d = din("idf16", [128, 128], fp16)
    out_d = nc.dram_tensor("out", [3, 128, R], fp16, kind="ExternalOutput").ap()

    with tile.TileContext(nc) as tc:
        with (
            tc.tile_pool(name="wts", bufs=1) as wts,
            tc.tile_pool(name="big", bufs=1) as big,
            tc.tile_pool(name="xls", bufs=2) as xls,
            tc.tile_pool(name="hbuf", bufs=2) as hbuf,
            tc.tile_pool(name="sgbuf", bufs=1) as sgbuf,
            tc.tile_pool(name="crmb", bufs=2) as crmb,
            tc.tile_pool(name="mkb", bufs=2) as mkb,
            tc.tile_pool(name="rrb", bufs=2) as rrb,
            tc.tile_pool(name="obuf", bufs=2) as obuf,
            tc.tile_pool(name="st", bufs=1) as st,
            tc.tile_pool(name="ps_h", bufs=2, space="PSUM") as ps_h,
            tc.tile_pool(name="ps_s", bufs=2, space="PSUM") as ps_s,
            tc.tile_pool(name="ps_t", bufs=2, space="PSUM") as ps_t,
            tc.tile_pool(name="ps_m", bufs=2, space="PSUM") as ps_m,
        ):
            def ldt(dram, tiles, tag, dt=fp16):
                t = wts.tile([128, tiles, 128], dt, tag=tag)
                nc.sync.dma_start(t, dram.rearrange("t p m -> p t m"))
                return t

            def ld2(dram, shape, tag, dt=f32):
                t = wts.tile(shape, dt, tag=tag)
                nc.sync.dma_start(t, dram)
                return t

            w1h = ldt(w1h_d, 4, "w1h"); w1l = ldt(w1l_d, 4, "w1l")
            w2h = ld2(w2h_d, [128, 128], "w2h", fp16)
            w2l = ld2(w2l_d, [128, 128], "w2l", fp16)
            w3h = ldt(w3h_d, 4, "w3h"); w3l = ldt(w3l_d, 4, "w3l")
            wg1h = ldt(wg1h_d, 4, "wg1h"); wg1l = ldt(wg1l_d, 4, "wg1l")
            wg2h = ldt(wg2h_d, 4, "wg2h"); wg2l = ldt(wg2l_d, 4, "wg2l")
            wr1 = ldt(wr1_d, 4, "wr1")
            wr2 = ld2(wr2_d, [128, K], "wr2", fp16)
            b1 = ld2(b1_d, [128, 1], "b1"); b2 = ld2(b2_d, [128, 1], "b2")
            b3 = ld2(b3_d, [128, 4], "b3")
            bg1 = ld2(bg1_d, [128, 1], "bg1")
            bg2 = ld2(bg2_d, [128, 4], "bg2")
            br1 = ld2(br1_d, [128, 1], "br1")
            br2 = ld2(br2_d, [128, 3], "br2")
            coef = ld2(coef_d, [128, 2], "coef")
            idf32 = ld2(idf32_d, [128, 128], "idf32")
            idf16 = ld2(idf16_d, [128, 128], "idf16", fp16)

            xh = big.tile([128, 4, R], fp16, tag="xh")
            nc.sync.dma_start(xh, xh_d.rearrange("f p r -> p f r"))
            ebuf = big.tile([128, NCOL, 512], fp16, tag="ebuf")
            junkD = big.tile([128, 512], fp16, tag="junkD")
            ones16 = big.tile([128, 512], fp16, tag="ones16")
            nc.vector.memset(ones16, 1.0)
            junkA = big.tile([128, 512], fp16, tag="junkA")

            lo_g, hi_g, tmp_g, cnt_g, gek_g, gekn_g, ng_g = [], [], [], [], [], [], []
            t0n = st.tile([128, NCOL], f32, tag="t0n")
            mus = st.tile([128, NCOL], f32, tag="mus")
            u32 = mybir.dt.uint32
            for g in range(NG):
                for lst, nm, dt_ in (
                        (lo_g, "lo", f32), (hi_g, "hi", f32),
                        (tmp_g, "tmp", f32), (cnt_g, "cnt", f32),
                        (gek_g, "gek", u32), (gekn_g, "gekn", u32),
                        (ng_g, "ng", f32)):
                    lst.append(st.tile([128, GC], dt_, tag="%s%d" % (nm, g),
                                       name="%s%d" % (nm, g)))

            def evict_dual(psum, bias, dual):
                """relu(psum)+bias via ACT (hh) and DVE residual (hl)."""
                hh = hbuf.tile([128, CHUNK], fp16, tag="hh")
                nc.scalar.activation(hh, psum, Act.Relu, bias=bias)
                if not dual:
                    return hh, None
                hl = hbuf.tile([128, CHUNK], fp16, tag="hl")
                nc.vector.scalar_tensor_tensor(
                    hl, psum, 0.0, hh, op0=Alu.max, op1=Alu.subtract)
                return hh, hl

            def net3(stats, movs, psum):
                ops = []
                for (sh_, sl_), (mh, ml) in zip(stats, movs):
                    ops.append((sh_, mh))
                    if ml is not None:
                        ops.append((sh_, ml))
                    ops.append((sl_, mh))
                n = len(ops)
                for i, (sta, mov) in enumerate(ops):
                    nc.tensor.matmul(psum, lhsT=sta, rhs=mov,
                                     start=(i == 0), stop=(i == n - 1))

            # =============== phase A ===============
            def phase_a(ck):
                r0 = ck * CHUNK
                xhc = [xh[:, ft, r0:r0 + CHUNK] for ft in range(4)]
                xlt = xls.tile([128, 4, CHUNK], fp16, tag="xl")
                for ft in range(4):
                    nc.sync.dma_start(xlt[:, ft, :], xl_d[ft, :, r0:r0 + CHUNK])
                xmov = [(xhc[ft], xlt[:, ft, :]) for ft in range(4)]

                p = ps_h.tile([128, CHUNK], f32, tag="h")
                net3([(w1h[:, ki, :], w1l[:, ki, :]) for ki in range(4)],
                     xmov, p)
                h1h, h1l = evict_dual(p, b1, True)

                p = ps_h.tile([128, CHUNK], f32, tag="h")
                net3([(w2h, w2l)], [(h1h, h1l)], p)
                h2h, h2l = evict_dual(p, b2, True)

                sa = []
                for mt in range(4):
                    pw = ps_s.tile([128, CHUNK], f32, tag="s")
                    net3([(w3h[:, mt, :], w3l[:, mt, :])], [(h2h, h2l)], pw)
                    t = sgbuf.tile([128, CHUNK], f32, tag="sa%d" % mt,
                                   name="sa%d" % mt)
                    nc.scalar.activation(t, pw, Act.Sigmoid,
                                         bias=b3[:, mt:mt + 1])
                    sa.append(t)

                p = ps_h.tile([128, CHUNK], f32, tag="h")
                net3([(wg1h[:, ki, :], wg1l[:, ki, :]) for ki in range(4)],
                     xmov, p)
                g1h, g1l = evict_dual(p, bg1, True)

                ct = []
                for mt in range(4):
                    pw = ps_s.tile([128, CHUNK], f32, tag="s")
                    net3([(wg2h[:, mt, :], wg2l[:, mt, :])], [(g1h, g1l)], pw)
                    t = sgbuf.tile([128, CHUNK], f32, tag="sg%d" % (mt % 2),
                                   name="sg%d" % (mt % 2))
                    nc.scalar.activation(t, pw, Act.Sigmoid,
                                         bias=bg2[:, mt:mt + 1])
                    c = sgbuf.tile([128, CHUNK], f32, tag="c%d" % mt,
                                   name="c%d" % mt)
                    nc.vector.tensor_mul(c, sa[mt], t)
                    ct.append(c)

                for rt in range(4):
                    col = ck * 4 + rt
                    ptr = ps_t.tile([128, CHUNK], f32, tag="tr")
                    for mt in range(4):
                        nc.tensor.transpose(
                            ptr[:, mt * 128:(mt + 1) * 128],
                            ct[mt][:, rt * 128:(rt + 1) * 128], idf32)
                    crm = crmb.tile([128, CHUNK], f32, tag="crm")
                    nc.scalar.activation(crm, ptr, Act.Identity,
                                         accum_out=mus[:, col:col + 1])
                    nc.vector.tensor_scalar(
                        t0n[:, col:col + 1], mus[:, col:col + 1],
                        coef[:, 0:1], coef[:, 1:2],
                        op0=Alu.mult, op1=Alu.add)
                    nc.scalar.activation(
                        ebuf[:, col, :], crm, Act.Identity,
                        bias=t0n[:, col:col + 1])

            # =============== phase B ===============
            def phase_b_init(g):
                nc.vector.memset(lo_g[g], -W_WIN)
                nc.vector.memset(hi_g[g], W_WIN)

            def phase_b_iter(g, it):
                lo, hi, tmp = lo_g[g], hi_g[g], tmp_g[g]
                cnt, gek, gekn, ngt = cnt_g[g], gek_g[g], gekn_g[g], ng_g[g]
                nc.vector.tensor_add(tmp, lo, hi)
                nc.vector.tensor_scalar_mul(tmp, tmp, 0.5)   # tmp = mid
                if NA:
                    nc.vector.tensor_scalar_mul(
                        ngt[:, GC - NA:], tmp[:, GC - NA:], -1.0)
                for i in range(GC - NA):
                    col = g * GC + i
                    nc.vector.scalar_tensor_tensor(
                        junkD, ebuf[:, col, :], tmp[:, i:i + 1], ones16,
                        op0=Alu.is_ge, op1=Alu.mult,
                        accum_out=cnt[:, i:i + 1])
                for i in range(GC - NA, GC):
                    col = g * GC + i
                    nc.scalar.activation(
                        junkA, ebuf[:, col, :], Act.Sign,
                        bias=ngt[:, i:i + 1],
                        accum_out=cnt[:, i:i + 1])
                nc.vector.tensor_scalar(
                    gek[:, :GC - NA], cnt[:, :GC - NA], float(K), None,
                    op0=Alu.is_ge)
                nc.vector.tensor_scalar(
                    gekn[:, :GC - NA], cnt[:, :GC - NA], float(K), None,
                    op0=Alu.is_lt)
                if NA:
                    thv = float(2 * K - 512)
                    nc.vector.tensor_scalar(
                        gek[:, GC - NA:], cnt[:, GC - NA:], thv, None,
                        op0=Alu.is_ge)
                    nc.vector.tensor_scalar(
                        gekn[:, GC - NA:], cnt[:, GC - NA:], thv, None,
                        op0=Alu.is_lt)
                nc.vector.copy_predicated(lo, gek, tmp)
                nc.vector.copy_predicated(hi, gekn, tmp)

            # =============== phase C ===============
            def phase_c(ck):
                g = ck // CPG
                r0 = ck * CHUNK
                mk = mkb.tile([128, 4, 512], fp16, tag="mk")
                for rt in range(4):
                    col = ck * 4 + rt
                    nc.vector.tensor_scalar(
                        mk[:, rt, :], ebuf[:, col, :],
                        lo_g[g][:, col - g * GC:col - g * GC + 1], None,
                        op0=Alu.is_ge)
                masked = []
                for ft in range(4):
                    pm = ps_m.tile([128, CHUNK], fp16, tag="pm")
                    for rt in range(4):
                        nc.tensor.transpose(
                            pm[:, rt * 128:(rt + 1) * 128],
                            mk[:, rt, ft * 128:(ft + 1) * 128], idf16)
                    mkd = mkb.tile([128, CHUNK], fp16, tag="mkd")
                    nc.vector.tensor_mul(mkd, pm, xh[:, ft, r0:r0 + CHUNK])
                    masked.append(mkd)

                p = ps_h.tile([128, CHUNK], f32, tag="h")
                for ft in range(4):
                    nc.tensor.matmul(p, lhsT=wr1[:, ft, :], rhs=masked[ft],
                                     start=(ft == 0), stop=(ft == 3))
                rr = rrb.tile([128, CHUNK], fp16, tag="rr")
                nc.scalar.activation(rr, p, Act.Relu, bias=br1)

                for (ot, o0, ow) in OSPLIT:
                    po = ps_h.tile([128, CHUNK], f32, tag="h")
                    nc.tensor.matmul(po[0:ow, :], lhsT=wr2[:, o0:o0 + ow],
                                     rhs=rr, start=True, stop=True)
                    of = obuf.tile([128, CHUNK], fp16, tag="of")
                    nc.scalar.activation(
                        of[0:ow, :], po[0:ow, :], Act.Identity,
                        bias=br2[0:ow, ot:ot + 1])
                    nc.sync.dma_start(out_d[ot, 0:ow, r0:r0 + CHUNK],
                                      of[0:ow, :])

            # =============== emission (software pipeline) ===============
            IT_SL = [(ITERS * s // CPG, ITERS * (s + 1) // CPG)
                     for s in range(CPG)]
            for g in range(NG):
                phase_b_init(g)
            for ck in range(NCHUNK):
                g, sl = ck // CPG, ck % CPG
                phase_a(ck)
                if g >= 1:
                    for it in range(*IT_SL[sl]):
                        phase_b_iter(g - 1, it)
                if g >= 2:
                    phase_c(CPG * (g - 2) + sl)
            for sl in range(CPG):
                for it in range(*IT_SL[sl]):
                    phase_b_iter(NG - 1, it)
                phase_c(CPG * (NG - 2) + sl)
            for sl in range(CPG):
                phase_c(CPG * (NG - 1) + sl)

    nc.compile()
    return nc


def kernel(**inputs):
    from concourse.bass_utils import run_bass_kernel_spmd

    x = np.asarray(inputs["x"], np.float32)
    names = ["W1", "b1", "W2", "b2", "W3", "b3", "Wg1", "bg1", "Wg2", "bg2",
             "Wr1", "br1", "Wr2", "br2"]
    P = {n: np.asarray(inputs[n], np.float32) for n in names}

    A1, C1 = _calibrate(x, P)

    def ksplit(Wm):
        h, l = _split16(Wm)
        return (np.ascontiguousarray(h.reshape(4, 128, 128)),
                np.ascontiguousarray(l.reshape(4, 128, 128)))

    def msplit(Wm):
        h, l = _split16(Wm)
        return (np.ascontiguousarray(h.reshape(128, 4, 128).transpose(1, 0, 2)),
                np.ascontiguousarray(l.reshape(128, 4, 128).transpose(1, 0, 2)))

    w1h, w1l = ksplit(P["W1"])
    w2h, w2l = _split16(P["W2"])
    w3h, w3l = msplit(P["W3"])
    wg1h, wg1l = ksplit(P["Wg1"])
    wg2h, wg2l = msplit(P["Wg2"])
    wr1 = np.ascontiguousarray(_f16(P["Wr1"]).reshape(4, 128, 128))
    wr2 = np.ascontiguousarray(_f16(P["Wr2"]))
    coef = np.zeros((128, 2), np.float32)
    coef[:, 0] = -A1 / 512.0
    coef[:, 1] = -C1
    b3p = np.zeros((128, 4), np.float32)
    b3p[:] = P["b3"].reshape(4, 128).T
    bg2p = np.zeros((128, 4), np.float32)
    bg2p[:] = P["bg2"].reshape(4, 128).T
    br2t = np.zeros(384, np.float32)
    br2t[:K] = P["br2"]
    br2p = np.ascontiguousarray(br2t.reshape(3, 128).T)
    ident = np.eye(128)
    shared = dict(
        w1h=w1h, w1l=w1l,
        w2h=np.ascontiguousarray(w2h), w2l=np.ascontiguousarray(w2l),
        w3h=w3h, w3l=w3l,
        wg1h=wg1h, wg1l=wg1l, wg2h=wg2h, wg2l=wg2l,
        wr1=wr1, wr2=wr2,
        b1=P["b1"].reshape(128, 1), b2=P["b2"].reshape(128, 1),
        b3=b3p, bg1=P["bg1"].reshape(128, 1), bg2=bg2p,
        br1=P["br1"].reshape(128, 1), br2=br2p,
        coef=coef,
        idf32=ident.astype(np.float32),
        idf16=ident.astype(np.float16),
    )

    in_maps = []
    for i in range(NCORES):
        xs = x[i * R:(i + 1) * R]
        xT = np.ascontiguousarray(xs.T)
        xTh = _f16(xT)
        xTl = _f16(xT - xTh.astype(np.float32))
        m = dict(shared)
        m["xh"] = np.ascontiguousarray(xTh.reshape(4, 128, R))
        m["xl"] = np.ascontiguousarray(xTl.reshape(4, 128, R))
        in_maps.append(m)

    if "nc" not in _cache:
        _cache["nc"] = _build_program()
    nc = _cache["nc"]
    _cache["in_maps"] = in_maps

    res = run_bass_kernel_spmd(nc, in_maps, list(range(NCORES)))
    outs = []
    for i in range(NCORES):
        o = res.results[i]["out"].astype(np.float32)   # [3,128,R]
        o = o.reshape(384, R)[:K]
        outs.append(np.ascontiguousarray(o.T))
    return np.concatenate(outs, axis=0)


if __name__ == "__main__":
    rng = np.random.default_rng(0)
    fake = {"x": rng.standard_normal((B, D), dtype=np.float32)}
    s = lambda f: 1.0 / np.sqrt(f)
    for nm, sh, fan in [("W1", (D, H), D), ("W2", (H, H), H), ("W3", (H, D), H),
                        ("Wg1", (D, H), D), ("Wg2", (H, D), H),
                        ("Wr1", (D, H), D), ("Wr2", (H, K), H)]:
        fake[nm] = rng.uniform(-s(fan), s(fan), sh).astype(np.float32)
    for nm, sh in [("b1", H), ("b2", H), ("b3", D), ("bg1", H), ("bg2", D),
                   ("br1", H), ("br2", K)]:
        fake[nm] = np.zeros(sh, np.float32)
    out = kernel(**fake)
    print("out", out.shape, out.dtype, float(np.abs(out).max()))


# revision 7
# speedup vs baseline: 1.4242x; 1.2781x over previous
"""Trainium2 Bass kernel for AdaptiveFeatureSelector (topk_masking).

v2: group-software-pipelined single pass.
 - Selector nets in 3-term fp16 split matmuls (hi/lo weights + dual-h /
   x-lo corrections) -> c = sigmoid*sigmoid in f32.
 - Per-row exact top-K=358: PE-transpose c to row-major (f32), per-row
   mean accumulated by the ACT eviction pass, affine-calibrated center
   t0, fp16 residuals, ITERS-step bisection with per-column counts
   (DVE tensor_scalar 4x + one ACT Sign column per group).
 - Mask = (resid >= lo), PE-transposed back, applied to xh, recon MLP,
   fp16 output (+br2 on device), un-transposed on host.
Phases are interleaved by group (A: selector+residuals, B: bisection,
C: mask+recon) so PE/ACT/DVE overlap.
"""

import sys

sys.path.insert(0, "/opt/trn_rl_repo")
import numpy as np

D = 512
H = 128
K = 358
B = 65536
NCORES = 8
R = B // NCORES
CHUNK = 512
NCHUNK = R // CHUNK      # 16
NCOL = R // 128          # 64
NG = 4
GC = NCOL // NG          # 16 cols per group
CPG = NCHUNK // NG       # 4 chunks per group
W_WIN = 0.015
ITERS = 11
NA = 7                   # ACT-counted cols per group (rest DVE)
OSPLIT = [(0, 0, 128), (1, 128, 128), (2, 256, 102)]

_cache = {}


def _f16(a):
    return np.asarray(a, np.float16)


def _split16(a):
    hi = _f16(a)
    lo = _f16(np.asarray(a, np.float32) - hi.astype(np.float32))
    return hi, lo


def _sig(a):
    return 1.0 / (1.0 + np.exp(-a))


def _calibrate(x, P):
    """Simulate the device c-pipeline on 512 rows; fit thr ~ A*mu + C."""
    xs = np.asarray(x[:512], np.float32)
    xh = _f16(xs)
    xl = _f16(xs - xh.astype(np.float32))

    def mm3(ah, al, Wm):
        wh, wl = _split16(Wm)
        out = ah.astype(np.float32) @ wh.astype(np.float32)
        out = out + ah.astype(np.float32) @ wl.astype(np.float32)
        if al is not None:
            out = out + al.astype(np.float32) @ wh.astype(np.float32)
        return out

    def ev(a):
        h = np.maximum(a, 0)
        hh = _f16(h)
        return hh, _f16(h - hh.astype(np.float32))

    h1h, h1l = ev(mm3(xh, xl, P["W1"]) + P["b1"])
    h2h, h2l = ev(mm3(h1h, h1l, P["W2"]) + P["b2"])
    imp = _sig(mm3(h2h, h2l, P["W3"]) + P["b3"])
    g1h, g1l = ev(mm3(xh, xl, P["Wg1"]) + P["bg1"])
    gate = _sig(mm3(g1h, g1l, P["Wg2"]) + P["bg2"])
    c = (imp * gate).astype(np.float32)
    mu = c.mean(1)
    thr = np.partition(c, D - K, axis=1)[:, D - K]
    A1, C1 = np.polyfit(mu, thr, 1)
    return float(A1), float(C1)


def _build_program():
    from concourse import bacc, mybir, tile

    f32 = mybir.dt.float32
    fp16 = mybir.dt.float16
    Act = mybir.ActivationFunctionType
    Alu = mybir.AluOpType

    nc = bacc.Bacc("TRN2", target_bir_lowering=False, debug=False,
                   num_devices=NCORES)

    def din(name, shape, dt=fp16):
        return nc.dram_tensor(name, shape, dt, kind="ExternalInput").ap()

    xh_d = din("xh", [4, 128, R])
    xl_d = din("xl", [4, 128, R])
    w1h_d = din("w1h", [4, 128, 128]); w1l_d = din("w1l", [4, 128, 128])
    w2h_d = din("w2h", [128, 128]);    w2l_d = din("w2l", [128, 128])
    w3h_d = din("w3h", [4, 128, 128]); w3l_d = din("w3l", [4, 128, 128])
    wg1h_d = din("wg1h", [4, 128, 128]); wg1l_d = din("wg1l", [4, 128, 128])
    wg2h_d = din("wg2h", [4, 128, 128]); wg2l_d = din("wg2l", [4, 128, 128])
    wr1_d = din("wr1", [4, 128, 128])
    wr2_d = din("wr2", [128, K])
    b1_d = din("b1", [128, 1], f32)
    b2_d = din("b2", [128, 1], f32)
    b3_d = din("b3", [128, 4], f32)
    bg1_d = din("bg1", [128, 1], f32)
    bg2_d = din("bg2", [128, 4], f32)
    br1_d = din("br1", [128, 1], f32)
    br2_d = din("br2", [128, 3], f32)
    coef_d = din("coef", [128, 2], f32)
    idf32_d = din("idf32", [128, 128], f32)
    idf16_d = din("idf16", [128, 128], fp16)
    out_d = nc.dram_tensor("out", [3, 128, R], fp16, kind="ExternalOutput").ap()

    with tile.TileContext(nc) as tc:
        with (
            tc.tile_pool(name="wts", bufs=1) as wts,
            tc.tile_pool(name="big", bufs=1) as big,
            tc.tile_pool(name="xls", bufs=2) as xls,
            tc.tile_pool(name="hbuf", bufs=2) as hbuf,
            tc.tile_pool(name="sgbuf", bufs=1) as sgbuf,
            tc.tile_pool(name="crmb", bufs=2) as crmb,
            tc.tile_pool(name="mkb", bufs=2) as mkb,
            tc.tile_pool(name="rrb", bufs=2) as rrb,
            tc.tile_pool(name="obuf", bufs=2) as obuf,
            tc.tile_pool(name="st", bufs=1) as st,
            tc.tile_pool(name="ps_h", bufs=2, space="PSUM") as ps_h,
            tc.tile_pool(name="ps_s", bufs=2, space="PSUM") as ps_s,
            tc.tile_pool(name="ps_t", bufs=2, space="PSUM") as ps_t,
            tc.tile_pool(name="ps_m", bufs=2, space="PSUM") as ps_m,
        ):
            def ldt(dram, tiles, tag, dt=fp16):
                t = wts.tile([128, tiles, 128], dt, tag=tag)
                nc.sync.dma_start(t, dram.rearrange("t p m -> p t m"))
                return t

            def ld2(dram, shape, tag, dt=f32):
                t = wts.tile(shape, dt, tag=tag)
                nc.sync.dma_start(t, dram)
                return t

            w1h = ldt(w1h_d, 4, "w1h"); w1l = ldt(w1l_d, 4, "w1l")
            w2h = ld2(w2h_d, [128, 128], "w2h", fp16)
            w2l = ld2(w2l_d, [128, 128], "w2l", fp16)
            w3h = ldt(w3h_d, 4, "w3h"); w3l = ldt(w3l_d, 4, "w3l")
            wg1h = ldt(wg1h_d, 4, "wg1h"); wg1l = ldt(wg1l_d, 4, "wg1l")
            wg2h = ldt(wg2h_d, 4, "wg2h"); wg2l = ldt(wg2l_d, 4, "wg2l")
            wr1 = ldt(wr1_d, 4, "wr1")
            wr2 = ld2(wr2_d, [128, K], "wr2", fp16)
            b1 = ld2(b1_d, [128, 1], "b1"); b2 = ld2(b2_d, [128, 1], "b2")
            b3 = ld2(b3_d, [128, 4], "b3")
            bg1 = ld2(bg1_d, [128, 1], "bg1")
            bg2 = ld2(bg2_d, [128, 4], "bg2")
            br1 = ld2(br1_d, [128, 1], "br1")
            br2 = ld2(br2_d, [128, 3], "br2")
            coef = ld2(coef_d, [128, 2], "coef")
            idf32 = ld2(idf32_d, [128, 128], "idf32")
            idf16 = ld2(idf16_d, [128, 128], "idf16", fp16)

            xh = big.tile([128, 4, R], fp16, tag="xh")
            nc.sync.dma_start(xh, xh_d.rearrange("f p r -> p f r"))
            ebuf = big.tile([128, NCOL, 512], fp16, tag="ebuf")
            junkD = big.tile([128, 512], fp16, tag="junkD")
            ones16 = big.tile([128, 512], fp16, tag="ones16")
            nc.vector.memset(ones16, 1.0)
            junkA = big.tile([128, 512], fp16, tag="junkA")

            lo_g, hi_g, tmp_g, cnt_g, gek_g, gekn_g, ng_g = [], [], [], [], [], [], []
            t0n = st.tile([128, NCOL], f32, tag="t0n")
            mus = st.tile([128, NCOL], f32, tag="mus")
            u32 = mybir.dt.uint32
            for g in range(NG):
                for lst, nm, dt_ in (
                        (lo_g, "lo", f32), (hi_g, "hi", f32),
                        (tmp_g, "tmp", f32), (cnt_g, "cnt", f32),
                        (gek_g, "gek", u32), (gekn_g, "gekn", u32),
                        (ng_g, "ng", f32)):
                    lst.append(st.tile([128, GC], dt_, tag="%s%d" % (nm, g),
                                       name="%s%d" % (nm, g)))

            def evict_dual(psum, bias, dual):
                """relu(psum)+bias via ACT (hh) and DVE residual (hl)."""
                hh = hbuf.tile([128, CHUNK], fp16, tag="hh")
                nc.scalar.activation(hh, psum, Act.Relu, bias=bias)
                if not dual:
                    return hh, None
                hl = hbuf.tile([128, CHUNK], fp16, tag="hl")
                nc.vector.scalar_tensor_tensor(
                    hl, psum, 0.0, hh, op0=Alu.max, op1=Alu.subtract)
                return hh, hl

            def net3(stats, movs, psum):
                ops = []
                for (sh_, sl_), (mh, ml) in zip(stats, movs):
                    ops.append((sh_, mh))
                    if ml is not None:
                        ops.append((sh_, ml))
                    ops.append((sl_, mh))
                n = len(ops)
                for i, (sta, mov) in enumerate(ops):
                    nc.tensor.matmul(psum, lhsT=sta, rhs=mov,
                                     start=(i == 0), stop=(i == n - 1))

            # =============== phase A ===============
            def phase_a(ck):
                r0 = ck * CHUNK
                xhc = [xh[:, ft, r0:r0 + CHUNK] for ft in range(4)]
                xlt = xls.tile([128, 4, CHUNK], fp16, tag="xl")
                for ft in range(4):
                    nc.sync.dma_start(xlt[:, ft, :], xl_d[ft, :, r0:r0 + CHUNK])
                xmov = [(xhc[ft], xlt[:, ft, :]) for ft in range(4)]

                p = ps_h.tile([128, CHUNK], f32, tag="h")
                net3([(w1h[:, ki, :], w1l[:, ki, :]) for ki in range(4)],
                     xmov, p)
                h1h, h1l = evict_dual(p, b1, True)

                p = ps_h.tile([128, CHUNK], f32, tag="h")
                net3([(w2h, w2l)], [(h1h, h1l)], p)
                h2h, h2l = evict_dual(p, b2, True)

                sa = []
                for mt in range(4):
                    pw = ps_s.tile([128, CHUNK], f32, tag="s")
                    net3([(w3h[:, mt, :], w3l[:, mt, :])], [(h2h, h2l)], pw)
                    t = sgbuf.tile([128, CHUNK], f32, tag="sa%d" % mt,
                                   name="sa%d" % mt)
                    nc.scalar.activation(t, pw, Act.Sigmoid,
                                         bias=b3[:, mt:mt + 1])
                    sa.append(t)

                p = ps_h.tile([128, CHUNK], f32, tag="h")
                net3([(wg1h[:, ki, :], wg1l[:, ki, :]) for ki in range(4)],
                     xmov, p)
                g1h, g1l = evict_dual(p, bg1, True)

                ct = []
                for mt in range(4):
                    pw = ps_s.tile([128, CHUNK], f32, tag="s")
                    net3([(wg2h[:, mt, :], wg2l[:, mt, :])], [(g1h, g1l)], pw)
                    t = sgbuf.tile([128, CHUNK], f32, tag="sg%d" % (mt % 2),
                                   name="sg%d" % (mt % 2))
                    nc.scalar.activation(t, pw, Act.Sigmoid,
                                         bias=bg2[:, mt:mt + 1])
                    c = sgbuf.tile([128, CHUNK], f32, tag="c%d" % mt,
                                   name="c%d" % mt)
                    nc.vector.tensor_mul(c, sa[mt], t)
                    ct.append(c)

                for rt in range(4):
                    col = ck * 4 + rt
                    ptr = ps_t.tile([128, CHUNK], f32, tag="tr")
                    for mt in range(4):
                        nc.tensor.transpose(
                            ptr[:, mt * 128:(mt + 1) * 128],
                            ct[mt][:, rt * 128:(rt + 1) * 128], idf32)
                    crm = crmb.tile([128, CHUNK], f32, tag="crm")
                    nc.scalar.activation(crm, ptr, Act.Identity,
                                         accum_out=mus[:, col:col + 1])
                    nc.vector.tensor_scalar(
                        t0n[:, col:col + 1], mus[:, col:col + 1],
                        coef[:, 0:1], coef[:, 1:2],
                        op0=Alu.mult, op1=Alu.add)
                    nc.vector.tensor_scalar(
                        ebuf[:, col, :], crm, t0n[:, col:col + 1], None,
                        op0=Alu.add)

            # =============== phase B ===============
            def phase_b_init(g):
                nc.vector.memset(lo_g[g], -W_WIN)
                nc.vector.memset(hi_g[g], W_WIN)

            def phase_b_iter(g, it):
                lo, hi, tmp = lo_g[g], hi_g[g], tmp_g[g]
                cnt, gek, gekn, ngt = cnt_g[g], gek_g[g], gekn_g[g], ng_g[g]
                nc.vector.tensor_add(tmp, lo, hi)
                nc.vector.tensor_scalar_mul(tmp, tmp, 0.5)   # tmp = mid
                if NA:
                    nc.vector.tensor_scalar_mul(
                        ngt[:, GC - NA:], tmp[:, GC - NA:], -1.0)
                for i in range(GC - NA):
                    col = g * GC + i
                    nc.vector.scalar_tensor_tensor(
                        junkD, ebuf[:, col, :], tmp[:, i:i + 1], ones16,
                        op0=Alu.is_ge, op1=Alu.mult,
                        accum_out=cnt[:, i:i + 1])
                for i in range(GC - NA, GC):
                    col = g * GC + i
                    nc.scalar.activation(
                        junkA, ebuf[:, col, :], Act.Sign,
                        bias=ngt[:, i:i + 1],
                        accum_out=cnt[:, i:i + 1])
                nc.vector.tensor_scalar(
                    gek[:, :GC - NA], cnt[:, :GC - NA], float(K), None,
                    op0=Alu.is_ge)
                nc.vector.tensor_scalar(
                    gekn[:, :GC - NA], cnt[:, :GC - NA], float(K), None,
                    op0=Alu.is_lt)
                if NA:
                    thv = float(2 * K - 512)
                    nc.vector.tensor_scalar(
                        gek[:, GC - NA:], cnt[:, GC - NA:], thv, None,
                        op0=Alu.is_ge)
                    nc.vector.tensor_scalar(
                        gekn[:, GC - NA:], cnt[:, GC - NA:], thv, None,
                        op0=Alu.is_lt)
                nc.vector.copy_predicated(lo, gek, tmp)
                nc.vector.copy_predicated(hi, gekn, tmp)

            # =============== phase C ===============
            def phase_c(ck):
                g = ck // CPG
                r0 = ck * CHUNK
                mk = mkb.tile([128, 4, 512], fp16, tag="mk")
                for rt in range(4):
                    col = ck * 4 + rt
                    nc.vector.tensor_scalar(
                        mk[:, rt, :], ebuf[:, col, :],
                        lo_g[g][:, col - g * GC:col - g * GC + 1], None,
                        op0=Alu.is_ge)
                masked = []
                for ft in range(4):
                    pm = ps_m.tile([128, CHUNK], fp16, tag="pm")
                    for rt in range(4):
                        nc.tensor.transpose(
                            pm[:, rt * 128:(rt + 1) * 128],
                            mk[:, rt, ft * 128:(ft + 1) * 128], idf16)
                    mkd = mkb.tile([128, CHUNK], fp16, tag="mkd")
                    nc.vector.tensor_mul(mkd, pm, xh[:, ft, r0:r0 + CHUNK])
                    masked.append(mkd)

                p = ps_h.tile([128, CHUNK], f32, tag="h")
                for ft in range(4):
                    nc.tensor.matmul(p, lhsT=wr1[:, ft, :], rhs=masked[ft],
                                     start=(ft == 0), stop=(ft == 3))
                rr = rrb.tile([128, CHUNK], fp16, tag="rr")
                nc.scalar.activation(rr, p, Act.Relu, bias=br1)

                for (ot, o0, ow) in OSPLIT:
                    po = ps_h.tile([128, CHUNK], f32, tag="h")
                    nc.tensor.matmul(po[0:ow, :], lhsT=wr2[:, o0:o0 + ow],
                                     rhs=rr, start=True, stop=True)
                    of = obuf.tile([128, CHUNK], fp16, tag="of")
                    nc.scalar.activation(
                        of[0:ow, :], po[0:ow, :], Act.Identity,
                        bias=br2[0:ow, ot:ot + 1])
                    nc.sync.dma_start(out_d[ot, 0:ow, r0:r0 + CHUNK],
                                      of[0:ow, :])

            # =============== emission (software pipeline) ===============
            IT_SL = [(ITERS * s // CPG, ITERS * (s + 1) // CPG)
                     for s in range(CPG)]
            for g in range(NG):
                phase_b_init(g)
            for ck in range(NCHUNK):
                g, sl = ck // CPG, ck % CPG
                phase_a(ck)
                if g >= 1:
                    for it in range(*IT_SL[sl]):
                        phase_b_iter(g - 1, it)
                if g >= 2:
                    phase_c(CPG * (g - 2) + sl)
            for sl in range(CPG):
                for it in range(*IT_SL[sl]):
                    phase_b_iter(NG - 1, it)
                phase_c(CPG * (NG - 2) + sl)
            for sl in range(CPG):
                phase_c(CPG * (NG - 1) + sl)

    nc.compile()
    return nc


def kernel(**inputs):
    from concourse.bass_utils import run_bass_kernel_spmd

    x = np.asarray(inputs["x"], np.float32)
    names = ["W1", "b1", "W2", "b2", "W3", "b3", "Wg1", "bg1", "Wg2", "bg2",
             "Wr1", "br1", "Wr2", "br2"]
    P = {n: np.asarray(inputs[n], np.float32) for n in names}

    A1, C1 = _calibrate(x, P)

    def ksplit(Wm):
        h, l = _split16(Wm)
        return (np.ascontiguousarray(h.reshape(4, 128, 128)),
                np.ascontiguousarray(l.reshape(4, 128, 128)))

    def msplit(Wm):
        h, l = _split16(Wm)
        return (np.ascontiguousarray(h.reshape(128, 4, 128).transpose(1, 0, 2)),
                np.ascontiguousarray(l.reshape(128, 4, 128).transpose(1, 0, 2)))

    w1h, w1l = ksplit(P["W1"])
    w2h, w2l = _split16(P["W2"])
    w3h, w3l = msplit(P["W3"])
    wg1h, wg1l = ksplit(P["Wg1"])
    wg2h, wg2l = msplit(P["Wg2"])
    wr1 = np.ascontiguousarray(_f16(P["Wr1"]).reshape(4, 128, 128))
    wr2 = np.ascontiguousarray(_f16(P["Wr2"]))
    coef = np.zeros((128, 2), np.float32)
    coef[:, 0] = -A1 / 512.0
    coef[:, 1] = -C1
    b3p = np.zeros((128, 4), np.float32)
    b3p[:] = P["b3"].reshape(4, 128).T
    bg2p = np.zeros((128, 4), np.float32)
    bg2p[:] = P["bg2"].reshape(4, 128).T
    br2t = np.zeros(384, np.float32)
    br2t[:K] = P["br2"]
    br2p = np.ascontiguousarray(br2t.reshape(3, 128).T)
    ident = np.eye(128)
    shared = dict(
        w1h=w1h, w1l=w1l,
        w2h=np.ascontiguousarray(w2h), w2l=np.ascontiguousarray(w2l),
        w3h=w3h, w3l=w3l,
        wg1h=wg1h, wg1l=wg1l, wg2h=wg2h, wg2l=wg2l,
        wr1=wr1, wr2=wr2,
        b1=P["b1"].reshape(128, 1), b2=P["b2"].reshape(128, 1),
        b3=b3p, bg1=P["bg1"].reshape(128, 1), bg2=bg2p,
        br1=P["br1"].reshape(128, 1), br2=br2p,
        coef=coef,
        idf32=ident.astype(np.float32),
        idf16=ident.astype(np.float16),
    )

    in_maps = []
    for i in range(NCORES):
        xs = x[i * R:(i + 1) * R]
        xT = np.ascontiguousarray(xs.T)
        xTh = _f16(xT)
        xTl = _f16(xT - xTh.astype(np.float32))
        m = dict(shared)
        m["xh"] = np.ascontiguousarray(xTh.reshape(4, 128, R))
        m["xl"] = np.ascontiguousarray(xTl.reshape(4, 128, R))
        in_maps.append(m)

    if "nc" not in _cache:
        _cache["nc"] = _build_program()
    nc = _cache["nc"]
    _cache["in_maps"] = in_maps

    res = run_bass_kernel_spmd(nc, in_maps, list(range(NCORES)))
    outs = []
    for i in range(NCORES):
        o = res.results[i]["out"].astype(np.float32)   # [3,128,R]
        o = o.reshape(384, R)[:K]
        outs.append(np.ascontiguousarray(o.T))
    return np.concatenate(outs, axis=0)


if __name__ == "__main__":
    rng = np.random.default_rng(0)
    fake = {"x": rng.standard_normal((B, D), dtype=np.float32)}
    s = lambda f: 1.0 / np.sqrt(f)
    for nm, sh, fan in [("W1", (D, H), D), ("W2", (H, H), H), ("W3", (H, D), H),
                        ("Wg1", (D, H), D), ("Wg2", (H, D), H),
                        ("Wr1", (D, H), D), ("Wr2", (H, K), H)]:
        fake[nm] = rng.uniform(-s(fan), s(fan), sh).astype(np.float32)
    for nm, sh in [("b1", H), ("b2", H), ("b3", D), ("bg1", H), ("bg2", D),
                   ("br1", H), ("br2", K)]:
        fake[nm] = np.zeros(sh, np.float32)
    out = kernel(**fake)
    print("out", out.shape, out.dtype, float(np.abs(out).max()))


# revision 9
# speedup vs baseline: 1.4997x; 1.0530x over previous
"""Trainium2 Bass kernel for AdaptiveFeatureSelector (topk_masking).

v2: group-software-pipelined single pass.
 - Selector nets in 3-term fp16 split matmuls (hi/lo weights + dual-h /
   x-lo corrections) -> c = sigmoid*sigmoid in f32.
 - Per-row exact top-K=358: PE-transpose c to row-major (f32), per-row
   mean accumulated by the ACT eviction pass, affine-calibrated center
   t0, fp16 residuals, ITERS-step bisection with per-column counts
   split ~evenly between DVE (scalar_tensor_tensor+accum) and ACT
   (Sign+accum) - both run ~0.7us/[128,512] (accum forces 1x mode).
 - Mask = (resid >= lo), PE-transposed back, applied to xh, recon MLP,
   fp16 output (+br2 on device), un-transposed on host.
Phases are interleaved by group (A: selector+residuals, B: bisection,
C: mask+recon) so PE/ACT/DVE overlap.
"""

import sys

sys.path.insert(0, "/opt/trn_rl_repo")
import numpy as np

D = 512
H = 128
K = 358
B = 65536
NCORES = 8
R = B // NCORES
CHUNK = 512
NCHUNK = R // CHUNK      # 16
NCOL = R // 128          # 64
NG = 4
GC = NCOL // NG          # 16 cols per group
CPG = NCHUNK // NG       # 4 chunks per group
W_WIN = 0.0075
ITERS = 10
NA = 7                   # ACT-counted cols per group (rest DVE)
OSPLIT = [(0, 0, 128), (1, 128, 128), (2, 256, 102)]

_cache = {}


def _f16(a):
    return np.asarray(a, np.float16)


def _split16(a):
    hi = _f16(a)
    lo = _f16(np.asarray(a, np.float32) - hi.astype(np.float32))
    return hi, lo


def _sig(a):
    return 1.0 / (1.0 + np.exp(-a))


def _calibrate(x, P):
    """Simulate the device c-pipeline on 512 rows; fit thr ~ A*mu + C."""
    xs = np.asarray(x[:512], np.float32)
    xh = _f16(xs)
    xl = _f16(xs - xh.astype(np.float32))

    def mm3(ah, al, Wm):
        wh, wl = _split16(Wm)
        out = ah.astype(np.float32) @ wh.astype(np.float32)
        out = out + ah.astype(np.float32) @ wl.astype(np.float32)
        if al is not None:
            out = out + al.astype(np.float32) @ wh.astype(np.float32)
        return out

    def ev(a):
        h = np.maximum(a, 0)
        hh = _f16(h)
        return hh, _f16(h - hh.astype(np.float32))

    h1h, h1l = ev(mm3(xh, xl, P["W1"]) + P["b1"])
    h2h, h2l = ev(mm3(h1h, h1l, P["W2"]) + P["b2"])
    imp = _sig(mm3(h2h, h2l, P["W3"]) + P["b3"])
    g1h, g1l = ev(mm3(xh, xl, P["Wg1"]) + P["bg1"])
    gate = _sig(mm3(g1h, g1l, P["Wg2"]) + P["bg2"])
    c = (imp * gate).astype(np.float32)
    mu = c.mean(1)
    thr = np.partition(c, D - K, axis=1)[:, D - K]
    A1, C1 = np.polyfit(mu, thr, 1)
    return float(A1), float(C1)


def _build_program():
    from concourse import bacc, mybir, tile

    f32 = mybir.dt.float32
    fp16 = mybir.dt.float16
    Act = mybir.ActivationFunctionType
    Alu = mybir.AluOpType

    nc = bacc.Bacc("TRN2", target_bir_lowering=False, debug=False,
                   num_devices=NCORES)

    def din(name, shape, dt=fp16):
        return nc.dram_tensor(name, shape, dt, kind="ExternalInput").ap()

    xh_d = din("xh", [4, 128, R])
    xl_d = din("xl", [4, 128, R])
    w1h_d = din("w1h", [4, 128, 128]); w1l_d = din("w1l", [4, 128, 128])
    w2h_d = din("w2h", [128, 128]);    w2l_d = din("w2l", [128, 128])
    w3h_d = din("w3h", [4, 128, 128]); w3l_d = din("w3l", [4, 128, 128])
    wg1h_d = din("wg1h", [4, 128, 128]); wg1l_d = din("wg1l", [4, 128, 128])
    wg2h_d = din("wg2h", [4, 128, 128]); wg2l_d = din("wg2l", [4, 128, 128])
    wr1_d = din("wr1", [4, 128, 128])
    wr2_d = din("wr2", [128, K])
    b1_d = din("b1", [128, 1], f32)
    b2_d = din("b2", [128, 1], f32)
    b3_d = din("b3", [128, 4], f32)
    bg1_d = din("bg1", [128, 1], f32)
    bg2_d = din("bg2", [128, 4], f32)
    br1_d = din("br1", [128, 1], f32)
    br2_d = din("br2", [128, 3], f32)
    coef_d = din("coef", [128, 2], f32)
    idf32_d = din("idf32", [128, 128], f32)
    idf16_d = din("idf16", [128, 128], fp16)
    out_d = nc.dram_tensor("out", [3, 128, R], fp16, kind="ExternalOutput").ap()

    with tile.TileContext(nc) as tc:
        with (
            tc.tile_pool(name="wts", bufs=1) as wts,
            tc.tile_pool(name="big", bufs=1) as big,
            tc.tile_pool(name="xls", bufs=2) as xls,
            tc.tile_pool(name="hbuf", bufs=2) as hbuf,
            tc.tile_pool(name="sgbuf", bufs=1) as sgbuf,
            tc.tile_pool(name="crmb", bufs=2) as crmb,
            tc.tile_pool(name="mkb", bufs=2) as mkb,
            tc.tile_pool(name="rrb", bufs=2) as rrb,
            tc.tile_pool(name="obuf", bufs=2) as obuf,
            tc.tile_pool(name="st", bufs=1) as st,
            tc.tile_pool(name="ps_h", bufs=2, space="PSUM") as ps_h,
            tc.tile_pool(name="ps_s", bufs=2, space="PSUM") as ps_s,
            tc.tile_pool(name="ps_t", bufs=2, space="PSUM") as ps_t,
            tc.tile_pool(name="ps_m", bufs=2, space="PSUM") as ps_m,
        ):
            def ldt(dram, tiles, tag, dt=fp16):
                t = wts.tile([128, tiles, 128], dt, tag=tag)
                nc.sync.dma_start(t, dram.rearrange("t p m -> p t m"))
                return t

            def ld2(dram, shape, tag, dt=f32):
                t = wts.tile(shape, dt, tag=tag)
                nc.sync.dma_start(t, dram)
                return t

            w1h = ldt(w1h_d, 4, "w1h"); w1l = ldt(w1l_d, 4, "w1l")
            w2h = ld2(w2h_d, [128, 128], "w2h", fp16)
            w2l = ld2(w2l_d, [128, 128], "w2l", fp16)
            w3h = ldt(w3h_d, 4, "w3h"); w3l = ldt(w3l_d, 4, "w3l")
            wg1h = ldt(wg1h_d, 4, "wg1h"); wg1l = ldt(wg1l_d, 4, "wg1l")
            wg2h = ldt(wg2h_d, 4, "wg2h"); wg2l = ldt(wg2l_d, 4, "wg2l")
            wr1 = ldt(wr1_d, 4, "wr1")
            wr2 = ld2(wr2_d, [128, K], "wr2", fp16)
            b1 = ld2(b1_d, [128, 1], "b1"); b2 = ld2(b2_d, [128, 1], "b2")
            b3 = ld2(b3_d, [128, 4], "b3")
            bg1 = ld2(bg1_d, [128, 1], "bg1")
            bg2 = ld2(bg2_d, [128, 4], "bg2")
            br1 = ld2(br1_d, [128, 1], "br1")
            br2 = ld2(br2_d, [128, 3], "br2")
            coef = ld2(coef_d, [128, 2], "coef")
            idf32 = ld2(idf32_d, [128, 128], "idf32")
            idf16 = ld2(idf16_d, [128, 128], "idf16", fp16)

            xh = big.tile([128, 4, R], fp16, tag="xh")
            nc.sync.dma_start(xh, xh_d.rearrange("f p r -> p f r"))
            ebuf = big.tile([128, NCOL, 512], fp16, tag="ebuf")
            junkD = big.tile([128, 512], fp16, tag="junkD")
            ones16 = big.tile([128, 512], fp16, tag="ones16")
            nc.vector.memset(ones16, 1.0)
            junkA = big.tile([128, 512], fp16, tag="junkA")

            lo_g, hi_g, tmp_g, cnt_g, gek_g, gekn_g, ng_g = [], [], [], [], [], [], []
            t0n = st.tile([128, NCOL], f32, tag="t0n")
            mus = st.tile([128, NCOL], f32, tag="mus")
            u32 = mybir.dt.uint32
            for g in range(NG):
                for lst, nm, dt_ in (
                        (lo_g, "lo", f32), (hi_g, "hi", f32),
                        (tmp_g, "tmp", f32), (cnt_g, "cnt", f32),
                        (gek_g, "gek", u32), (gekn_g, "gekn", u32),
                        (ng_g, "ng", f32)):
                    lst.append(st.tile([128, GC], dt_, tag="%s%d" % (nm, g),
                                       name="%s%d" % (nm, g)))

            def evict_dual(psum, bias, dual):
                """relu(psum)+bias via ACT (hh) and DVE residual (hl)."""
                hh = hbuf.tile([128, CHUNK], fp16, tag="hh")
                nc.scalar.activation(hh, psum, Act.Relu, bias=bias)
                if not dual:
                    return hh, None
                hl = hbuf.tile([128, CHUNK], fp16, tag="hl")
                nc.vector.scalar_tensor_tensor(
                    hl, psum, 0.0, hh, op0=Alu.max, op1=Alu.subtract)
                return hh, hl

            def net3(stats, movs, psum):
                ops = []
                for (sh_, sl_), (mh, ml) in zip(stats, movs):
                    ops.append((sh_, mh))
                    if ml is not None:
                        ops.append((sh_, ml))
                    ops.append((sl_, mh))
                n = len(ops)
                for i, (sta, mov) in enumerate(ops):
                    nc.tensor.matmul(psum, lhsT=sta, rhs=mov,
                                     start=(i == 0), stop=(i == n - 1))

            # =============== phase A ===============
            def phase_a(ck):
                r0 = ck * CHUNK
                xhc = [xh[:, ft, r0:r0 + CHUNK] for ft in range(4)]
                xlt = xls.tile([128, 4, CHUNK], fp16, tag="xl")
                for ft in range(4):
                    nc.sync.dma_start(xlt[:, ft, :], xl_d[ft, :, r0:r0 + CHUNK])
                xmov = [(xhc[ft], xlt[:, ft, :]) for ft in range(4)]

                p = ps_h.tile([128, CHUNK], f32, tag="h")
                net3([(w1h[:, ki, :], w1l[:, ki, :]) for ki in range(4)],
                     xmov, p)
                h1h, h1l = evict_dual(p, b1, True)

                p = ps_h.tile([128, CHUNK], f32, tag="h")
                net3([(w2h, w2l)], [(h1h, h1l)], p)
                h2h, h2l = evict_dual(p, b2, True)

                sa = []
                for mt in range(4):
                    pw = ps_s.tile([128, CHUNK], f32, tag="s")
                    net3([(w3h[:, mt, :], w3l[:, mt, :])], [(h2h, h2l)], pw)
                    t = sgbuf.tile([128, CHUNK], f32, tag="sa%d" % mt,
                                   name="sa%d" % mt)
                    nc.scalar.activation(t, pw, Act.Sigmoid,
                                         bias=b3[:, mt:mt + 1])
                    sa.append(t)

                p = ps_h.tile([128, CHUNK], f32, tag="h")
                net3([(wg1h[:, ki, :], wg1l[:, ki, :]) for ki in range(4)],
                     xmov, p)
                g1h, g1l = evict_dual(p, bg1, True)

                ct = []
                for mt in range(4):
                    pw = ps_s.tile([128, CHUNK], f32, tag="s")
                    net3([(wg2h[:, mt, :], wg2l[:, mt, :])], [(g1h, g1l)], pw)
                    t = sgbuf.tile([128, CHUNK], f32, tag="sg%d" % (mt % 2),
                                   name="sg%d" % (mt % 2))
                    nc.scalar.activation(t, pw, Act.Sigmoid,
                                         bias=bg2[:, mt:mt + 1])
                    c = sgbuf.tile([128, CHUNK], f32, tag="c%d" % mt,
                                   name="c%d" % mt)
                    nc.vector.tensor_mul(c, sa[mt], t)
                    ct.append(c)

                for rt in range(4):
                    col = ck * 4 + rt
                    ptr = ps_t.tile([128, CHUNK], f32, tag="tr")
                    for mt in range(4):
                        nc.tensor.transpose(
                            ptr[:, mt * 128:(mt + 1) * 128],
                            ct[mt][:, rt * 128:(rt + 1) * 128], idf32)
                    crm = crmb.tile([128, CHUNK], f32, tag="crm")
                    nc.scalar.activation(crm, ptr, Act.Identity,
                                         accum_out=mus[:, col:col + 1])
                    nc.vector.tensor_scalar(
                        t0n[:, col:col + 1], mus[:, col:col + 1],
                        coef[:, 0:1], coef[:, 1:2],
                        op0=Alu.mult, op1=Alu.add)
                    nc.vector.tensor_scalar(
                        ebuf[:, col, :], crm, t0n[:, col:col + 1], None,
                        op0=Alu.add)

            # =============== phase B ===============
            def phase_b_init(g):
                nc.vector.memset(lo_g[g], -W_WIN)
                nc.vector.memset(hi_g[g], W_WIN)

            def phase_b_iter(g, it):
                lo, hi, tmp = lo_g[g], hi_g[g], tmp_g[g]
                cnt, gek, gekn, ngt = cnt_g[g], gek_g[g], gekn_g[g], ng_g[g]
                nc.vector.tensor_add(tmp, lo, hi)
                nc.vector.tensor_scalar_mul(tmp, tmp, 0.5)   # tmp = mid
                if NA:
                    nc.vector.tensor_scalar_mul(
                        ngt[:, GC - NA:], tmp[:, GC - NA:], -1.0)
                for i in range(GC - NA):
                    col = g * GC + i
                    nc.vector.scalar_tensor_tensor(
                        junkD, ebuf[:, col, :], tmp[:, i:i + 1], ones16,
                        op0=Alu.is_ge, op1=Alu.mult,
                        accum_out=cnt[:, i:i + 1])
                for i in range(GC - NA, GC):
                    col = g * GC + i
                    nc.scalar.activation(
                        junkA, ebuf[:, col, :], Act.Sign,
                        bias=ngt[:, i:i + 1],
                        accum_out=cnt[:, i:i + 1])
                nc.vector.tensor_scalar(
                    gek[:, :GC - NA], cnt[:, :GC - NA], float(K), None,
                    op0=Alu.is_ge)
                nc.vector.tensor_scalar(
                    gekn[:, :GC - NA], cnt[:, :GC - NA], float(K), None,
                    op0=Alu.is_lt)
                if NA:
                    thv = float(2 * K - 512)
                    nc.vector.tensor_scalar(
                        gek[:, GC - NA:], cnt[:, GC - NA:], thv, None,
                        op0=Alu.is_ge)
                    nc.vector.tensor_scalar(
                        gekn[:, GC - NA:], cnt[:, GC - NA:], thv, None,
                        op0=Alu.is_lt)
                nc.vector.copy_predicated(lo, gek, tmp)
                nc.vector.copy_predicated(hi, gekn, tmp)

            # =============== phase C ===============
            def phase_c(ck):
                g = ck // CPG
                r0 = ck * CHUNK
                mk = mkb.tile([128, 4, 512], fp16, tag="mk")
                for rt in range(4):
                    col = ck * 4 + rt
                    nc.vector.tensor_scalar(
                        mk[:, rt, :], ebuf[:, col, :],
                        lo_g[g][:, col - g * GC:col - g * GC + 1], None,
                        op0=Alu.is_ge)
                masked = []
                for ft in range(4):
                    pm = ps_m.tile([128, CHUNK], fp16, tag="pm")
                    for rt in range(4):
                        nc.tensor.transpose(
                            pm[:, rt * 128:(rt + 1) * 128],
                            mk[:, rt, ft * 128:(ft + 1) * 128], idf16)
                    mkd = mkb.tile([128, CHUNK], fp16, tag="mkd")
                    nc.vector.tensor_mul(mkd, pm, xh[:, ft, r0:r0 + CHUNK])
                    masked.append(mkd)

                p = ps_h.tile([128, CHUNK], f32, tag="h")
                for ft in range(4):
                    nc.tensor.matmul(p, lhsT=wr1[:, ft, :], rhs=masked[ft],
                                     start=(ft == 0), stop=(ft == 3))
                rr = rrb.tile([128, CHUNK], fp16, tag="rr")
                nc.scalar.activation(rr, p, Act.Relu, bias=br1)

                for (ot, o0, ow) in OSPLIT:
                    po = ps_h.tile([128, CHUNK], f32, tag="h")
                    nc.tensor.matmul(po[0:ow, :], lhsT=wr2[:, o0:o0 + ow],
                                     rhs=rr, start=True, stop=True)
                    of = obuf.tile([128, CHUNK], fp16, tag="of")
                    nc.scalar.activation(
                        of[0:ow, :], po[0:ow, :], Act.Identity,
                        bias=br2[0:ow, ot:ot + 1])
                    nc.sync.dma_start(out_d[ot, 0:ow, r0:r0 + CHUNK],
                                      of[0:ow, :])

            # =============== emission (software pipeline) ===============
            IT_SL = [(ITERS * s // CPG, ITERS * (s + 1) // CPG)
                     for s in range(CPG)]
            for g in range(NG):
                phase_b_init(g)
            for ck in range(NCHUNK):
                g, sl = ck // CPG, ck % CPG
                phase_a(ck)
                if g >= 1:
                    for it in range(*IT_SL[sl]):
                        phase_b_iter(g - 1, it)
                if g >= 2:
                    phase_c(CPG * (g - 2) + sl)
            for sl in range(CPG):
                for it in range(*IT_SL[sl]):
                    phase_b_iter(NG - 1, it)
                phase_c(CPG * (NG - 2) + sl)
            for sl in range(CPG):
                phase_c(CPG * (NG - 1) + sl)

    nc.compile()
    return nc


def kernel(**inputs):
    from concourse.bass_utils import run_bass_kernel_spmd

    x = np.asarray(inputs["x"], np.float32)
    names = ["W1", "b1", "W2", "b2", "W3", "b3", "Wg1", "bg1", "Wg2", "bg2",
             "Wr1", "br1", "Wr2", "br2"]
    P = {n: np.asarray(inputs[n], np.float32) for n in names}

    A1, C1 = _calibrate(x, P)

    def ksplit(Wm):
        h, l = _split16(Wm)
        return (np.ascontiguousarray(h.reshape(4, 128, 128)),
                np.ascontiguousarray(l.reshape(4, 128, 128)))

    def msplit(Wm):
        h, l = _split16(Wm)
        return (np.ascontiguousarray(h.reshape(128, 4, 128).transpose(1, 0, 2)),
                np.ascontiguousarray(l.reshape(128, 4, 128).transpose(1, 0, 2)))

    w1h, w1l = ksplit(P["W1"])
    w2h, w2l = _split16(P["W2"])
    w3h, w3l = msplit(P["W3"])
    wg1h, wg1l = ksplit(P["Wg1"])
    wg2h, wg2l = msplit(P["Wg2"])
    wr1 = np.ascontiguousarray(_f16(P["Wr1"]).reshape(4, 128, 128))
    wr2 = np.ascontiguousarray(_f16(P["Wr2"]))
    coef = np.zeros((128, 2), np.float32)
    coef[:, 0] = -A1 / 512.0
    coef[:, 1] = -C1
    b3p = np.zeros((128, 4), np.float32)
    b3p[:] = P["b3"].reshape(4, 128).T
    bg2p = np.zeros((128, 4), np.float32)
    bg2p[:] = P["bg2"].reshape(4, 128).T
    br2t = np.zeros(384, np.float32)
    br2t[:K] = P["br2"]
    br2p = np.ascontiguousarray(br2t.reshape(3, 128).T)
    ident = np.eye(128)
    shared = dict(
        w1h=w1h, w1l=w1l,
        w2h=np.ascontiguousarray(w2h), w2l=np.ascontiguousarray(w2l),
        w3h=w3h, w3l=w3l,
        wg1h=wg1h, wg1l=wg1l, wg2h=wg2h, wg2l=wg2l,
        wr1=wr1, wr2=wr2,
        b1=P["b1"].reshape(128, 1), b2=P["b2"].reshape(128, 1),
        b3=b3p, bg1=P["bg1"].reshape(128, 1), bg2=bg2p,
        br1=P["br1"].reshape(128, 1), br2=br2p,
        coef=coef,
        idf32=ident.astype(np.float32),
        idf16=ident.astype(np.float16),
    )

    in_maps = []
    for i in range(NCORES):
        xs = x[i * R:(i + 1) * R]
        xT = np.ascontiguousarray(xs.T)
        xTh = _f16(xT)
        xTl = _f16(xT - xTh.astype(np.float32))
        m = dict(shared)
        m["xh"] = np.ascontiguousarray(xTh.reshape(4, 128, R))
        m["xl"] = np.ascontiguousarray(xTl.reshape(4, 128, R))
        in_maps.append(m)

    if "nc" not in _cache:
        _cache["nc"] = _build_program()
    nc = _cache["nc"]
    _cache["in_maps"] = in_maps

    res = run_bass_kernel_spmd(nc, in_maps, list(range(NCORES)))
    outs = []
    for i in range(NCORES):
        o = res.results[i]["out"].astype(np.float32)   # [3,128,R]
        o = o.reshape(384, R)[:K]
        outs.append(np.ascontiguousarray(o.T))
    return np.concatenate(outs, axis=0)


if __name__ == "__main__":
    rng = np.random.default_rng(0)
    fake = {"x": rng.standard_normal((B, D), dtype=np.float32)}
    s = lambda f: 1.0 / np.sqrt(f)
    for nm, sh, fan in [("W1", (D, H), D), ("W2", (H, H), H), ("W3", (H, D), H),
                        ("Wg1", (D, H), D), ("Wg2", (H, D), H),
                        ("Wr1", (D, H), D), ("Wr2", (H, K), H)]:
        fake[nm] = rng.uniform(-s(fan), s(fan), sh).astype(np.float32)
    for nm, sh in [("b1", H), ("b2", H), ("b3", D), ("bg1", H), ("bg2", D),
                   ("br1", H), ("br2", K)]:
        fake[nm] = np.zeros(sh, np.float32)
    out = kernel(**fake)
    print("out", out.shape, out.dtype, float(np.abs(out).max()))
